# revision 44
# baseline (speedup 1.0000x reference)
"""Trainium2 Bass kernel for a 4-layer dense transformer (AKT-style).

Sharding: data-parallel over batch. B=8 batch elements -> 1 per NeuronCore.
Each core runs the full 4-layer stack on its own (S=1024, D=512) slice with
no collectives; weights are replicated.

Per-core layout: feature-major activations xT [D, S] (partition dim = feature
tiles of 128). Matmuls run in bf16 (fp32 PSUM accumulation). Attention uses
the symmetric-scores trick: S = kq @ kq^T is symmetric, so a [j, i]-layout
strip of scores doubles as the transposed-probabilities operand after a
strictly-upper-triangular causal mask; softmax denominators come from an
extra ones-column appended to V, and the per-(head, i) normalizer is applied
after the PV matmul via a DRAM-bounce partition broadcast.
"""
import sys

sys.path.insert(0, "/opt/trn_rl_repo")

import math

import ml_dtypes
import numpy as np

import concourse.bass as bass
import concourse.tile as tile
from concourse import bacc, mybir
from concourse.bass_utils import run_bass_kernel_spmd

F32 = mybir.dt.float32
BF16 = mybir.dt.bfloat16
AF = mybir.ActivationFunctionType
ALU = mybir.AluOpType

B, S, D, H, FF, L = 8, 1024, 512, 8, 2048, 4
DK = D // H          # 64
NKT = D // 128       # 4  feature tiles
NJT = S // 128       # 8  token tiles
NFT = FF // 128      # 16 ffn tiles
SCALE = 1.0 / math.sqrt(DK)
EPS = 1e-5
NCORES = 8

_PROG_CACHE = {}


def _strip_chunks(a):
    """Column chunks (absolute i ranges) for scores/PV strip of j-tile a:
    the 128-wide diagonal block first, then pieces that cross neither an
    absolute 512-boundary (PV psum banks) nor a strip-local one (scores
    psum banks, local = absolute - 128*a)."""
    chunks = [(128 * a, 128 * a + 128)]
    start = 128 * a + 128
    pts = sorted({512, 128 * a + 512, S})
    for p in pts:
        if start < p <= S:
            chunks.append((start, p))
            start = p
    return chunks


def _build(has_bv, bk_zero=True, ln1_triv=True, ln2_triv=True):
    nc = bacc.Bacc("TRN2", target_bir_lowering=False, debug=False,
                   num_devices=NCORES)

    qT = nc.declare_dram_parameter("qT", [D, S], F32, isOutput=False)
    qaT = nc.declare_dram_parameter("qaT", [D, S], F32, isOutput=False)
    peT = nc.declare_dram_parameter("peT", [D, S], F32, isOutput=False)
    wk_e = nc.declare_dram_parameter("wkT", [L, D, D], BF16, isOutput=False)
    wv_e = nc.declare_dram_parameter("wvT", [L, D, D], BF16, isOutput=False)
    wo_e = nc.declare_dram_parameter("woT", [L, D, D], BF16, isOutput=False)
    w1_e = nc.declare_dram_parameter("w1T", [L, D, FF], BF16, isOutput=False)
    w2_e = nc.declare_dram_parameter("w2T", [L, FF, D], BF16, isOutput=False)
    # per-feature params packed [128, L, ntiles]
    bk_e = nc.declare_dram_parameter("bkp", [128, L, NKT], F32, isOutput=False)
    bo_e = nc.declare_dram_parameter("bop", [128, L, NKT], F32, isOutput=False)
    b1_e = nc.declare_dram_parameter("b1p", [128, L, NFT], F32, isOutput=False)
    b2_e = nc.declare_dram_parameter("b2p", [128, L, NKT], F32, isOutput=False)
    l1s_e = nc.declare_dram_parameter("l1s", [128, L, NKT], F32, isOutput=False)
    l1b_e = nc.declare_dram_parameter("l1b", [128, L, NKT], F32, isOutput=False)
    l2s_e = nc.declare_dram_parameter("l2s", [128, L, NKT], F32, isOutput=False)
    l2b_e = nc.declare_dram_parameter("l2b", [128, L, NKT], F32, isOutput=False)
    bv_e = nc.declare_dram_parameter("bvp", [1, L, D], BF16, isOutput=False) if has_bv else None
    mask_e = nc.declare_dram_parameter("mask01", [128, 128], BF16, isOutput=False)
    out_e = nc.declare_dram_parameter("outT", [D, S], F32, isOutput=True)

    with tile.TileContext(nc) as tc:
        with (
            tc.tile_pool(name="res", bufs=1) as res,         # resident activations
            tc.tile_pool(name="wqkv", bufs=2) as wqkv,       # per-layer D x D weights
            tc.tile_pool(name="wff", bufs=1) as wff,         # per-layer ffn weights
            tc.tile_pool(name="ld", bufs=4) as ld,           # input staging
            tc.tile_pool(name="pt", bufs=4) as ptp,          # exp'd prob strips
            tc.tile_pool(name="vp", bufs=2) as vp,           # v_ext double buffer
            tc.tile_pool(name="hs", bufs=1) as hs,           # per-head stats rows
            tc.tile_pool(name="bc", bufs=1) as bc,           # LN broadcast tiles
            tc.tile_pool(name="bch", bufs=1) as bch,
            tc.tile_pool(name="oc", bufs=2) as ocp,         # head broadcast tiles
            tc.tile_pool(name="dr", bufs=4, space="DRAM") as dr,
            tc.tile_pool(name="ps", bufs=3, space="PSUM") as ps,
            tc.tile_pool(name="psh", bufs=2, space="PSUM") as psh,
        ):
            # ---- constants & params
            mask01 = res.tile([128, 128], BF16, tag="mask")
            nc.gpsimd.dma_start(out=mask01, in_=mask_e[:])
            ones128 = res.tile([128, 128], BF16, tag="ones")
            nc.vector.memset(ones128, 1.0)
            if has_bv:
                ones_row = res.tile([1, 128], BF16, tag="onesr")
                nc.vector.memset(ones_row, 1.0)
                bv_sb = res.tile([1, L, D], BF16, tag="bv")
                nc.gpsimd.dma_start(out=bv_sb, in_=bv_e[:])
            params = {}
            for name, ext, nt in (("bk", bk_e, NKT), ("bo", bo_e, NKT),
                                  ("b1", b1_e, NFT), ("b2", b2_e, NKT),
                                  ("l1s", l1s_e, NKT), ("l1b", l1b_e, NKT),
                                  ("l2s", l2s_e, NKT), ("l2b", l2b_e, NKT)):
                t = res.tile([128, L, nt], F32, tag="prm_" + name)
                nc.gpsimd.dma_start(out=t, in_=ext[:])
                params[name] = t

            # ---- residents
            xT = res.tile([128, NKT, S], F32, tag="xT")
            x_bf = res.tile([128, NKT, S], BF16, tag="x_bf")
            y_bf = res.tile([128, NKT, S], BF16, tag="y_bf")
            kq_bf = res.tile([128, NKT, S], BF16, tag="kq_bf")
            outcat = res.tile([128, NKT, S], BF16, tag="outcat")
            h1_bf = res.tile([128, NFT, S], BF16, tag="h1_bf")
            xsq_bf = res.tile([128, NKT, S], BF16, tag="xsq")

            # ---- x = q + pe ; y = qa + pe  (staged per feature tile)
            qT4 = qT.rearrange("(k p) s -> p k s", p=128)
            qaT4 = qaT.rearrange("(k p) s -> p k s", p=128)
            peT4 = peT.rearrange("(k p) s -> p k s", p=128)
            for kt in range(NKT):
                for ch in range(2):
                    cs = slice(ch * 512, ch * 512 + 512)
                    pe_t = ld.tile([128, 512], F32, tag="ld")
                    nc.sync.dma_start(out=pe_t, in_=peT4[:, kt, cs])
                    q_t = ld.tile([128, 512], F32, tag="ld")
                    nc.sync.dma_start(out=q_t, in_=qT4[:, kt, cs])
                    qa_t = ld.tile([128, 512], F32, tag="ld")
                    nc.sync.dma_start(out=qa_t, in_=qaT4[:, kt, cs])
                    nc.vector.tensor_add(xT[:, kt, cs], q_t, pe_t)
                    nc.gpsimd.tensor_copy(x_bf[:, kt, cs], xT[:, kt, cs])
                    nc.vector.tensor_add(y_bf[:, kt, cs], qa_t, pe_t)

            def layernorm(lname_s, lname_b, li, triv, refresh=True):
                """In-place LN over features of xT; refresh x_bf. Sums/stats/
                apply run as two independent 512-column pipelines; the Ln+Exp
                rstd step runs once over both halves to avoid ACT table
                reloads. Sums via ones128 lhsT come out pre-broadcast."""
                meanb = bc.tile([128, S], F32, tag="meanb")
                sb = bc.tile([128, S], F32, tag="statb")  # ex2 -> ve -> rstd
                for ch in range(2):
                    c0 = ch * 512
                    cs = slice(c0, c0 + 512)
                    for kt in range(NKT):
                        if refresh:
                            nc.scalar.copy(x_bf[:, kt, cs], xT[:, kt, cs])
                        nc.scalar.activation(out=xsq_bf[:, kt, cs],
                                             in_=xT[:, kt, cs], func=AF.Square)
                    lp0 = psh.tile([128, 512], F32, tag="mmh")
                    lp1 = psh.tile([128, 512], F32, tag="mmh")
                    for kt in range(NKT):
                        nc.tensor.matmul(lp0, lhsT=ones128, rhs=x_bf[:, kt, cs],
                                         start=(kt == 0), stop=(kt == NKT - 1))
                    for kt in range(NKT):
                        nc.tensor.matmul(lp1, lhsT=ones128, rhs=xsq_bf[:, kt, cs],
                                         start=(kt == 0), stop=(kt == NKT - 1))
                    nc.vector.tensor_scalar_mul(meanb[:, cs], lp0, 1.0 / D)
                    nc.vector.tensor_scalar_mul(sb[:, cs], lp1, 1.0 / D)
                    nc.vector.tensor_mul(lp0, meanb[:, cs], meanb[:, cs])
                    nc.vector.scalar_tensor_tensor(out=sb[:, cs], in0=sb[:, cs],
                                                   scalar=float(EPS), in1=lp0,
                                                   op0=ALU.add, op1=ALU.subtract)
                    nc.scalar.activation(out=sb[:, cs], in_=sb[:, cs], func=AF.Ln)
                    nc.scalar.activation(out=sb[:, cs], in_=sb[:, cs],
                                         func=AF.Exp, scale=-0.5)
                for ch in range(2):
                    cs = slice(ch * 512, ch * 512 + 512)
                    for kt in range(NKT):
                        nc.vector.tensor_sub(xT[:, kt, cs], xT[:, kt, cs],
                                             meanb[:, cs])
                        nc.vector.tensor_mul(xT[:, kt, cs], xT[:, kt, cs],
                                             sb[:, cs])
                        if not triv:
                            nc.vector.tensor_scalar(
                                out=xT[:, kt, cs], in0=xT[:, kt, cs],
                                scalar1=params[lname_s][:, li, kt:kt + 1],
                                scalar2=params[lname_b][:, li, kt:kt + 1],
                                op0=ALU.mult, op1=ALU.add)
                        if refresh:
                            nc.scalar.copy(x_bf[:, kt, cs], xT[:, kt, cs])

            def vproj(li, wv, jts):
                """v = y @ WvT (token-major) -- depends only on y_bf/wv, so it
                can fill PE bubbles in other phases (attention exp waits)."""
                vx4 = _vx4_of[li]
                nc.gpsimd.memset(vx4[:, jts.start:jts.stop, :, 64:65], 1.0)
                for jt in jts:
                    pp = ps.tile([128, S], F32, tag="mm")
                    for kt in range(NKT):
                        nc.tensor.matmul(pp[:, 0:512],
                                         lhsT=y_bf[:, kt, jt * 128:jt * 128 + 128],
                                         rhs=wv[:, kt, :],
                                         start=(kt == 0), stop=(kt == NKT - 1))
                    if has_bv:
                        nc.tensor.matmul(pp[:, 0:512], lhsT=ones_row,
                                         rhs=bv_sb[0:1, li, :],
                                         start=False, stop=True,
                                         skip_group_check=True)
                    nc.scalar.copy(
                        vx4[:, jt, :, 0:64],
                        pp[:, 0:512].rearrange("p (h c) -> p h c", c=64))
                return vx4

            _vx4_of = {}

            def new_vext(li):
                vext = vp.tile([128, NJT, H * 72], BF16, tag="vext")
                _vx4_of[li] = vext.rearrange("p j (h c) -> p j h c", c=72)
                return _vx4_of[li]

            wv_tiles = {}

            def load_wv(li):
                t = wqkv.tile([128, NKT, D], BF16, tag="wv")
                nc.sync.dma_start(out=t, in_=wv_e[li].rearrange("(k p) m -> p k m", p=128))
                return t

            wv_tiles[0] = load_wv(0)
            new_vext(0)
            vx4 = vproj(0, wv_tiles[0], range(0, NJT))

            for li in range(L):
                wk = wqkv.tile([128, NKT, D], BF16, tag="wk")
                nc.sync.dma_start(out=wk, in_=wk_e[li].rearrange("(k p) m -> p k m", p=128))
                wo = wqkv.tile([128, NKT, D], BF16, tag="wo")
                nc.sync.dma_start(out=wo, in_=wo_e[li].rearrange("(k p) m -> p k m", p=128))
                w1 = wff.tile([128, NKT, FF], BF16, tag="w1")
                nc.sync.dma_start(out=w1, in_=w1_e[li].rearrange("(k p) m -> p k m", p=128))
                w2 = wff.tile([128, NFT, D], BF16, tag="w2")
                nc.sync.dma_start(out=w2, in_=w2_e[li].rearrange("(k p) m -> p k m", p=128))

                # ---- kq projection (feature-major out) : kqT = WkT.T @ xT
                for mt in range(NKT):
                    pp = ps.tile([128, S], F32, tag="mm")
                    for ch in range(2):
                        c0 = ch * 512
                        for kt in range(NKT):
                            nc.tensor.matmul(pp[:, c0:c0 + 512],
                                             lhsT=wk[:, kt, mt * 128:mt * 128 + 128],
                                             rhs=x_bf[:, kt, c0:c0 + 512],
                                             start=(kt == 0), stop=(kt == NKT - 1))
                    if bk_zero:
                        nc.scalar.copy(kq_bf[:, mt, :], pp)
                    else:
                        nc.scalar.activation(out=kq_bf[:, mt, :], in_=pp,
                                             func=AF.Identity,
                                             bias=params["bk"][:, li, mt:mt + 1])

                # ---- attention, head by head; after each head, emit one
                # next-layer v-proj token tile as PE filler for the exp waits
                for h in range(H):
                    po = (h % 2) * 64
                    kqh = kq_bf[po:po + 64, h // 2, :]
                    acc0 = psh.tile([128, 512], F32, tag="mmh")
                    acc1 = psh.tile([128, 512], F32, tag="mmh")
                    # strip groups share one psum tile + one exp: strips
                    # 4..7 are narrow enough to pack pairwise with no padding
                    # (local offsets chosen so no matmul crosses a psum bank)
                    def emit_group(group, locs):
                        sc = ps.tile([128, S], F32, tag="mm")
                        for a in group:
                            for (s0, e0) in _strip_chunks(a):
                                lo = locs[a] + s0 - 128 * a
                                nc.tensor.matmul(sc[:, lo:lo + (e0 - s0)],
                                                 lhsT=kqh[:, 128 * a:128 * a + 128],
                                                 rhs=kqh[:, s0:s0 + (e0 - s0)],
                                                 start=True, stop=True)
                        wtot = max(locs[a] + S - 128 * a for a in group)
                        pt = ptp.tile([128, S], BF16, tag="pt")
                        nc.scalar.activation(out=pt[:, 0:wtot], in_=sc[:, 0:wtot],
                                             func=AF.Exp, scale=float(SCALE))
                        if (len(group) == 2
                                and locs[group[1]] - locs[group[0]] <= 512):
                            stride = locs[group[1]] - locs[group[0]]
                            view = pt[:, 0:stride * len(group)].rearrange(
                                "p (g c) -> p g c", c=stride)
                            nc.vector.tensor_mul(view[:, :, 0:128], view[:, :, 0:128],
                                                 mask01[:, None, :].to_broadcast(
                                                     [128, len(group), 128]))
                        else:
                            for a in group:
                                lo = locs[a]
                                nc.vector.tensor_mul(pt[:, lo:lo + 128],
                                                     pt[:, lo:lo + 128], mask01)
                        return pt

                    def emit_pv(group, locs, pt):
                        for a in group:
                            for (s0, e0) in _strip_chunks(a):
                                acc = acc0 if s0 < 512 else acc1
                                o0 = s0 - (0 if s0 < 512 else 512)
                                lo = locs[a] + s0 - 128 * a
                                nc.tensor.matmul(acc[0:65, o0:o0 + (e0 - s0)],
                                                 lhsT=vx4[:, a, h, 0:65],
                                                 rhs=pt[:, lo:lo + (e0 - s0)],
                                                 start=(a == 0),
                                                 stop=(e0 == 128 * a + 128),
                                                 skip_group_check=True)

                    GROUPS = [([0], {0: 0}), ([1], {1: 0}),
                              ([2, 6], {2: 0, 6: 768}),
                              ([3, 7], {3: 0, 7: 640}),
                              ([4, 5], {4: 0, 5: 512})]
                    prev = None
                    for group, locs in GROUPS:
                        pt = emit_group(group, locs)
                        if prev is not None:
                            emit_pv(*prev)
                        prev = (group, locs, pt)
                    emit_pv(*prev)
                    l_sb = hs.tile([1, S], F32, tag="lsum")
                    recip = hs.tile([1, S], F32, tag="recip")
                    nc.vector.tensor_copy(l_sb[:, 0:512], acc0[64:65, :])
                    nc.vector.tensor_copy(l_sb[:, 512:1024], acc1[64:65, :])
                    # evict the unnormalized PV output right away so the psum
                    # slots free for the next head; normalize from SBUF later
                    ocraw = ocp.tile([64, S], F32, tag="ocraw")
                    nc.vector.tensor_copy(ocraw[:, 0:512], acc0[0:64, :])
                    nc.vector.tensor_copy(ocraw[:, 512:1024], acc1[0:64, :])
                    nc.vector.memset(l_sb[:, 0:1], 1.0)
                    nc.vector.reciprocal(recip, l_sb)
                    rbh = bch.tile([64, S], F32, tag="rbh")
                    bnc = dr.tile([1, S], F32, tag="bounce")
                    nc.sync.dma_start(out=bnc, in_=recip)
                    bap = bnc[:]
                    nc.sync.dma_start(out=rbh, in_=bass.AP(
                        tensor=bap.tensor, offset=bap.offset,
                        ap=[[0, 64]] + bap.ap[1:]))
                    nc.vector.tensor_mul(outcat[po:po + 64, h // 2, :],
                                         ocraw, rbh)

                # ---- out projection + residual into xT
                for mt in range(NKT):
                    pp = ps.tile([128, S], F32, tag="mm")
                    for ch in range(2):
                        c0 = ch * 512
                        for kt in range(NKT):
                            nc.tensor.matmul(pp[:, c0:c0 + 512],
                                             lhsT=wo[:, kt, mt * 128:mt * 128 + 128],
                                             rhs=outcat[:, kt, c0:c0 + 512],
                                             start=(kt == 0), stop=(kt == NKT - 1))
                    nc.vector.scalar_tensor_tensor(
                        out=xT[:, mt, :], in0=pp,
                        scalar=params["bo"][:, li, mt:mt + 1],
                        in1=xT[:, mt, :], op0=ALU.add, op1=ALU.add)

                if li + 1 < L:
                    wv_tiles[li + 1] = load_wv(li + 1)
                    new_vext(li + 1)
                    vproj(li + 1, wv_tiles[li + 1], range(0, NJT // 2))
                layernorm("l1s", "l1b", li, ln1_triv)

                # ---- ffn1: h1 = relu(W1 @ x + b1), feature-major
                for mt in range(NFT):
                    pp = ps.tile([128, S], F32, tag="mm")
                    for ch in range(2):
                        c0 = ch * 512
                        for kt in range(NKT):
                            nc.tensor.matmul(pp[:, c0:c0 + 512],
                                             lhsT=w1[:, kt, mt * 128:mt * 128 + 128],
                                             rhs=x_bf[:, kt, c0:c0 + 512],
                                             start=(kt == 0), stop=(kt == NKT - 1))
                    nc.scalar.activation(out=h1_bf[:, mt, :], in_=pp,
                                         func=AF.Relu,
                                         bias=params["b1"][:, li, mt:mt + 1])

                # ---- ffn2 + residual into xT
                for mt in range(NKT):
                    pp = ps.tile([128, S], F32, tag="mm")
                    for ch in range(2):
                        c0 = ch * 512
                        for kt in range(NFT):
                            nc.tensor.matmul(pp[:, c0:c0 + 512],
                                             lhsT=w2[:, kt, mt * 128:mt * 128 + 128],
                                             rhs=h1_bf[:, kt, c0:c0 + 512],
                                             start=(kt == 0), stop=(kt == NFT - 1))
                    nc.vector.scalar_tensor_tensor(
                        out=xT[:, mt, :], in0=pp,
                        scalar=params["b2"][:, li, mt:mt + 1],
                        in1=xT[:, mt, :], op0=ALU.add, op1=ALU.add)

                if li + 1 < L:
                    vproj(li + 1, wv_tiles[li + 1], range(NJT // 2, NJT))
                layernorm("l2s", "l2b", li, ln2_triv)
                if li + 1 < L:
                    vx4 = _vx4_of[li + 1]

            oute4 = out_e.rearrange("(k p) s -> p k s", p=128)
            for ch in range(2):
                cs = slice(ch * 512, ch * 512 + 512)
                nc.sync.dma_start(out=oute4[:, :, cs], in_=xT[:, :, cs])

    # Pin every ACT instruction to the one table set that contains all the
    # functions this kernel uses (Exp/Ln/Identity/Relu/Square/Copy), so the
    # whole kernel needs a single ACT_TABLE_LOAD instead of thrashing between
    # the exp- and ln-anchored sets on every layernorm. Indices are preserved
    # (the pass emits act_func_set_id by list position).
    import concourse.bacc as _bacc_mod
    _orig_gat = _bacc_mod.get_activation_tables
    def _pinned_tables(arch):
        tabs = _orig_gat(arch)
        return {name: (funcs if name == "natural_log_exp_and_others" else set())
                for name, funcs in tabs.items()}
    _bacc_mod.get_activation_tables = _pinned_tables
    try:
        nc.compile()
    finally:
        _bacc_mod.get_activation_tables = _orig_gat
    return nc


def _pack_feat(arr, nt):
    """(L, nt*128) fp32 -> [128, L, nt]"""
    Ld = arr.shape[0]
    return np.ascontiguousarray(arr.reshape(Ld, nt, 128).transpose(2, 0, 1)).astype(np.float32)


def kernel(q_embed_data, qa_embed_data, pe, Wk, bk, Wv, bv, Wo, bo,
           ln1_s, ln1_b, W1, b1, W2, b2, ln2_s, ln2_b, **_unused):
    q = np.asarray(q_embed_data, np.float32)
    qa = np.asarray(qa_embed_data, np.float32)
    pe = np.asarray(pe, np.float32)
    bf = ml_dtypes.bfloat16

    has_bv = bool(np.any(np.asarray(bv)))
    bk_zero = not bool(np.any(np.asarray(bk)))
    ln1_triv = bool(np.all(np.asarray(ln1_s) == 1.0) and not np.any(np.asarray(ln1_b)))
    ln2_triv = bool(np.all(np.asarray(ln2_s) == 1.0) and not np.any(np.asarray(ln2_b)))
    key = (has_bv, bk_zero, ln1_triv, ln2_triv)
    if key not in _PROG_CACHE:
        _PROG_CACHE[key] = _build(has_bv, bk_zero, ln1_triv, ln2_triv)
    nc = _PROG_CACHE[key]

    shared = {
        "peT": np.ascontiguousarray(pe.T),
        "wkT": np.ascontiguousarray(np.asarray(Wk, np.float32).transpose(0, 2, 1)).astype(bf),
        "wvT": np.ascontiguousarray(np.asarray(Wv, np.float32).transpose(0, 2, 1)).astype(bf),
        "woT": np.ascontiguousarray(np.asarray(Wo, np.float32).transpose(0, 2, 1)).astype(bf),
        "w1T": np.ascontiguousarray(np.asarray(W1, np.float32).transpose(0, 2, 1)).astype(bf),
        "w2T": np.ascontiguousarray(np.asarray(W2, np.float32).transpose(0, 2, 1)).astype(bf),
        "bkp": _pack_feat(np.asarray(bk, np.float32), NKT),
        "bop": _pack_feat(np.asarray(bo, np.float32), NKT),
        "b1p": _pack_feat(np.asarray(b1, np.float32), NFT),
        "b2p": _pack_feat(np.asarray(b2, np.float32), NKT),
        "l1s": _pack_feat(np.asarray(ln1_s, np.float32), NKT),
        "l1b": _pack_feat(np.asarray(ln1_b, np.float32), NKT),
        "l2s": _pack_feat(np.asarray(ln2_s, np.float32), NKT),
        "l2b": _pack_feat(np.asarray(ln2_b, np.float32), NKT),
        "mask01": (np.arange(128)[:, None] < np.arange(128)[None, :]).astype(bf),
    }
    if has_bv:
        shared["bvp"] = np.asarray(bv, np.float32).reshape(1, L, D).astype(bf)

    in_maps = []
    for c in range(NCORES):
        m = dict(shared)
        m["qT"] = np.ascontiguousarray(q[c].T)
        m["qaT"] = np.ascontiguousarray(qa[c].T)
        in_maps.append(m)

    res = run_bass_kernel_spmd(nc, in_maps, core_ids=list(range(NCORES)))
    out = np.stack([np.ascontiguousarray(res.results[c]["outT"].T)
                    for c in range(NCORES)])
    return out.astype(np.float32)
